# revision 5
# baseline (speedup 1.0000x reference)
"""Multi-head self-attention (B=4, S=2048, D=1024, H=16) on 8 trn2 NeuronCores.

Sharding: core c -> batch b = c//2, head-group g = c%2 (8 heads, 512 of the
1024 output/QKV columns). Each core computes Q/K/V projections for its slice
and full attention for its 8 heads. Host does layout prep (x transpose + bf16
cast, W column slices) and the final gather/transpose - no collectives needed.

All matmuls in bf16 (psum accumulation f32): full PE rate, half the weight-load
time and DMA of f32r, and lower PE power draw (the f32r version tripped the HW
utilization throttle to ~54%).

Per-core device pipeline:
  phase 1: one pass over x chunks (loaded once): QT[512,2048], KT[512,2048]
           bf16 = (W.T @ xT-chunks); V[2048,512] bf16 packed into
           Vx[128,16,8,65] with a ones column per head (PV denominator).
  phase 2: per head-pair, per q-chunk(512): software-pipelined over 16
           k-blocks:
             scoresT[k,q] psum[128,1024] <- KT-tile.T @ QT-chunk (2 heads,
               one bank each, tile_position rows 0/64);
             one ACT exp over both banks -> ex bf16 [128,1024];
             pv[65,512] psum += Vx-tile.T @ ex-half (row 64 = denominator),
               issued one k-block behind the scores so PE/ACT overlap.
           normalize: out = pv[0:64] * partition_broadcast(recip(pv[64])).
  output: outT[512,2048] f32 per core; host writes out[b,:,cols] = outT.T.
"""
import numpy as np
import ml_dtypes

import concourse.bacc as bacc
import concourse.mybir as mybir
import concourse.tile as tile
from concourse.bass_utils import run_bass_kernel_spmd

B, S, D, H = 4, 2048, 1024, 16
DH = D // H            # 64
NCORES = 8
HLOC = H // 2          # 8 heads per core
DLOC = HLOC * DH       # 512 output cols per core
F32 = mybir.dt.float32
BF16 = mybir.dt.bfloat16
EXPF = mybir.ActivationFunctionType.Exp

SC = 512               # s-chunk in phase 1
QC = 512               # q-chunk in phase 2
NKB = S // 128         # 16 k-blocks
NDT = D // 128         # 8 contraction tiles for QKV


def _build():
    nc = bacc.Bacc("TRN2", target_bir_lowering=False, debug=False, num_devices=NCORES)
    xT = nc.dram_tensor("xT", [D, S], BF16, kind="ExternalInput").ap()
    Wq = nc.dram_tensor("Wq", [D, DLOC], BF16, kind="ExternalInput").ap()
    Wk = nc.dram_tensor("Wk", [D, DLOC], BF16, kind="ExternalInput").ap()
    Wv = nc.dram_tensor("Wv", [D, DLOC], BF16, kind="ExternalInput").ap()
    out = nc.dram_tensor("outT", [DLOC, S], F32, kind="ExternalOutput").ap()

    xT_t = xT.rearrange("(o p) s -> p o s", p=128)        # [128, 8, 2048]
    out_t = out.rearrange("(o p) s -> p o s", p=128)      # [128, 4, 2048]

    with tile.TileContext(nc) as tc:
        with tc.tile_pool(name="persist", bufs=1) as keep:
            qt = keep.tile([128, DLOC // 128, S], BF16)   # QT  [p, 4, 2048]
            kt = keep.tile([128, DLOC // 128, S], BF16)   # KT  [p, 4, 2048]
            vx = keep.tile([128, NKB, HLOC, DH + 1], BF16)

            # ---------------- phase 1: QKV projections -------------------
            with nc.named_scope("qkv"), \
                 tc.tile_pool(name="p1w", bufs=1) as p1w, \
                 tc.tile_pool(name="p1x", bufs=2) as p1x, \
                 tc.tile_pool(name="p1ps", bufs=4, space="PSUM") as p1ps:
                wq_sb = p1w.tile([128, NDT, DLOC], BF16)
                wk_sb = p1w.tile([128, NDT, DLOC], BF16)
                wv_sb = p1w.tile([128, NDT, DLOC], BF16)
                nc.gpsimd.dma_start(wq_sb[:], Wq.rearrange("(o p) m -> p o m", p=128))
                nc.gpsimd.dma_start(wk_sb[:], Wk.rearrange("(o p) m -> p o m", p=128))
                nc.gpsimd.dma_start(wv_sb[:], Wv.rearrange("(o p) m -> p o m", p=128))
                ones_t = p1w.tile([128, NKB, HLOC], BF16)
                nc.vector.memset(ones_t[:], 1.0)
                nc.vector.tensor_copy(vx[:, :, :, DH], ones_t[:])

                for sc in range(S // SC):
                    xc = p1x.tile([128, NDT, SC], BF16, tag="xc", name=f"xc{sc}")
                    nc.gpsimd.dma_start(xc[:], xT_t[:, :, sc * SC:(sc + 1) * SC])
                    for w_sb, dst in ((wk_sb, kt), (wq_sb, qt)):
                        for m in range(DLOC // 128):
                            ps = p1ps.tile([128, SC], F32, tag="qk")
                            for dt_i in range(NDT):
                                nc.tensor.matmul(
                                    ps[:],
                                    w_sb[:, dt_i, m * 128:(m + 1) * 128],
                                    xc[:, dt_i, :],
                                    start=(dt_i == 0), stop=(dt_i == NDT - 1),
                                )
                            nc.vector.tensor_copy(
                                dst[:, m, sc * SC:(sc + 1) * SC], ps[:])
                    for sb in range(SC // 128):
                        ps = p1ps.tile([128, DLOC], F32, tag="qk")
                        for dt_i in range(NDT):
                            nc.tensor.matmul(
                                ps[:],
                                xc[:, dt_i, sb * 128:(sb + 1) * 128],
                                wv_sb[:, dt_i, :],
                                start=(dt_i == 0), stop=(dt_i == NDT - 1),
                            )
                        s_idx = sc * (SC // 128) + sb
                        nc.vector.tensor_copy(
                            vx[:, s_idx, :, 0:DH],
                            ps[:].rearrange("p (h d) -> p h d", h=HLOC))

            # ---------------- phase 2: attention -------------------------
            with nc.named_scope("attn"), \
                 tc.tile_pool(name="p2o", bufs=1) as p2o, \
                 tc.tile_pool(name="p2e", bufs=4) as p2e, \
                 tc.tile_pool(name="p2n", bufs=2) as p2n, \
                 tc.tile_pool(name="ps_s", bufs=2, space="PSUM") as ps_s, \
                 tc.tile_pool(name="ps_pv", bufs=2, space="PSUM") as ps_pv:
                ot = p2o.tile([128, DLOC // 128, S], F32)

                for hp in range(HLOC // 2):
                    for qc in range(S // QC):
                        qs = slice(qc * QC, (qc + 1) * QC)
                        pvs = [ps_pv.tile([DH + 1, QC], F32, tag=f"pv{h}",
                                          name=f"pv{h}") for h in range(2)]
                        exs = [None] * NKB

                        def emit_pv(kb):
                            for h in range(2):
                                nc.tensor.matmul(
                                    pvs[h][:], vx[:, kb, 2 * hp + h, :],
                                    exs[kb][:, h, :],
                                    start=(kb == 0), stop=(kb == NKB - 1),
                                    skip_group_check=True)

                        for kb in range(NKB):
                            ks = slice(kb * 128, (kb + 1) * 128)
                            spp = ps_s.tile([128, 2, QC], F32, tag="sc",
                                            name=f"sp{kb % 2}")
                            for h in range(2):
                                nc.tensor.matmul(
                                    spp[:, h, :],
                                    kt[64 * h:64 * h + 64, hp, ks],
                                    qt[64 * h:64 * h + 64, hp, qs],
                                    start=True, stop=True,
                                    tile_position=(64 * h, 0))
                            if kb > 0:
                                emit_pv(kb - 1)
                            ex = p2e.tile([128, 2, QC], BF16, tag="ex",
                                          name=f"ex{kb % 4}")
                            nc.scalar.activation(ex[:], spp[:], EXPF,
                                                 scale=1.0 / H)
                            exs[kb] = ex
                        emit_pv(NKB - 1)

                        for h in range(2):
                            dr = p2n.tile([1, QC], F32, tag="dr", name="dr")
                            nc.vector.tensor_copy(dr[:], pvs[h][DH:DH + 1, :])
                            den = p2n.tile([1, QC], F32, tag="den", name="den")
                            nc.vector.reciprocal_approx_fast(den[:], dr[:])
                            bc = p2n.tile([DH, QC], F32, tag="bc", name="bc")
                            nc.gpsimd.partition_broadcast(bc[:], den[:])
                            nc.vector.tensor_mul(
                                ot[64 * h:64 * h + 64, hp, qs],
                                pvs[h][0:DH, :], bc[:])
                    nc.gpsimd.dma_start(out_t[:, hp, :], ot[:, hp, :])

    nc.compile()
    return nc


def run(inputs, trace=False):
    x = np.asarray(inputs["encoder_input"], dtype=np.float32)
    Wq = np.asarray(inputs["Wq"], dtype=np.float32)
    Wk = np.asarray(inputs["Wk"], dtype=np.float32)
    Wv = np.asarray(inputs["Wv"], dtype=np.float32)

    nc = _build()
    bf = ml_dtypes.bfloat16
    in_maps = []
    for c in range(NCORES):
        b, g = c // 2, c % 2
        cols = slice(g * DLOC, (g + 1) * DLOC)
        in_maps.append({
            "xT": np.ascontiguousarray(x[b].T).astype(bf),
            "Wq": np.ascontiguousarray(Wq[:, cols]).astype(bf),
            "Wk": np.ascontiguousarray(Wk[:, cols]).astype(bf),
            "Wv": np.ascontiguousarray(Wv[:, cols]).astype(bf),
        })
    res = run_bass_kernel_spmd(nc, in_maps, core_ids=list(range(NCORES)),
                               trace=trace)
    out = np.empty((B, S, D), dtype=np.float32)
    for c in range(NCORES):
        b, g = c // 2, c % 2
        out[b, :, g * DLOC:(g + 1) * DLOC] = res.results[c]["outT"].T
    return out, res


def kernel(**inputs):
    out, _ = run(inputs, trace=False)
    return out


# revision 6
# speedup vs baseline: 1.0525x; 1.0525x over previous
"""Multi-head self-attention (B=4, S=2048, D=1024, H=16) on 8 trn2 NeuronCores.

Sharding: core c -> batch b = c//2, head-group g = c%2 (8 heads, 512 of the
1024 output/QKV columns). Each core computes Q/K/V projections for its slice
and full attention for its 8 heads. Host does layout prep (x transpose + bf16
cast, W column slices) and the final gather/transpose - no collectives needed.

All matmuls in bf16 (psum accumulation f32): full PE rate, half the weight-load
time and DMA of f32r, and lower PE power draw (the f32r version tripped the HW
utilization throttle to ~54% duty).

Single fused pipeline per core (projections threaded into attention):
  prefix: V for all s (packed into Vx[128,16,8,65] bf16 with a ones column
          per head for the PV denominator), then K/Q for head-pair 0.
  per head-pair hp, per q-chunk(512): software-pipelined over 16 k-blocks:
      scoresT[k,q] psum[128,1024] <- KT-tile.T @ QT-chunk (2 heads, one bank
        each, tile_position rows 0/64 - the pair executes concurrently);
      one ACT exp over both banks -> ex bf16 [128,1024];
      pv[65,512] psum += Vx-tile.T @ ex-half (row 64 = denominator), issued
        one k-block behind the scores so PE overlaps ACT;
      every 8th iteration: one K/Q projection group for head-pair hp+1 is
        emitted (PE fills its exp-stall slack; ACT never starves).
    normalize: out = pv[0:64] * partition_broadcast(recip(pv[64])).
  output: outT[512,2048] f32 per core; host writes out[b,:,cols] = outT.T.
"""
import numpy as np
import ml_dtypes

import concourse.bacc as bacc
import concourse.mybir as mybir
import concourse.tile as tile
from concourse.bass_utils import run_bass_kernel_spmd

B, S, D, H = 4, 2048, 1024, 16
DH = D // H            # 64
NCORES = 8
HLOC = H // 2          # 8 heads per core
DLOC = HLOC * DH       # 512 output cols per core
NM = DLOC // 128       # 4 head-pair blocks
F32 = mybir.dt.float32
BF16 = mybir.dt.bfloat16
EXPF = mybir.ActivationFunctionType.Exp

SC = 512               # s-chunk for projections
QC = 512               # q-chunk in attention
NKB = S // 128         # 16 k-blocks
NDT = D // 128         # 8 contraction tiles for QKV


def _build():
    nc = bacc.Bacc("TRN2", target_bir_lowering=False, debug=False, num_devices=NCORES)
    xT = nc.dram_tensor("xT", [D, S], BF16, kind="ExternalInput").ap()
    Wq = nc.dram_tensor("Wq", [D, DLOC], BF16, kind="ExternalInput").ap()
    Wk = nc.dram_tensor("Wk", [D, DLOC], BF16, kind="ExternalInput").ap()
    Wv = nc.dram_tensor("Wv", [D, DLOC], BF16, kind="ExternalInput").ap()
    out = nc.dram_tensor("outT", [DLOC, S], F32, kind="ExternalOutput").ap()

    xT_t = xT.rearrange("(o p) s -> p o s", p=128)        # [128, 8, 2048]
    out_t = out.rearrange("(o p) s -> p o s", p=128)      # [128, 4, 2048]

    with tile.TileContext(nc) as tc:
        with tc.tile_pool(name="persist", bufs=1) as keep, \
             tc.tile_pool(name="p2e", bufs=4) as p2e, \
             tc.tile_pool(name="p2n", bufs=2) as p2n, \
             tc.tile_pool(name="p1ps", bufs=2, space="PSUM") as p1ps, \
             tc.tile_pool(name="ps_s", bufs=2, space="PSUM") as ps_s, \
             tc.tile_pool(name="ps_pv", bufs=1, space="PSUM") as ps_pv:
            qts = [keep.tile([128, S], BF16, name=f"qt{m}") for m in range(NM)]
            kts = [keep.tile([128, S], BF16, name=f"kt{m}") for m in range(NM)]
            vx = keep.tile([128, NKB, HLOC, DH + 1], BF16)
            ot = keep.tile([128, NM, S], F32)
            wq_sb = keep.tile([128, NDT, DLOC], BF16)
            wk_sb = keep.tile([128, NDT, DLOC], BF16)
            wv_sb = keep.tile([128, NDT, DLOC], BF16)
            xall = keep.tile([128, NDT, S], BF16)

            # DMA order: interleave x chunks and weights for earliest V start.
            nc.gpsimd.dma_start(xall[:, :, 0:SC], xT_t[:, :, 0:SC])
            nc.gpsimd.dma_start(wv_sb[:], Wv.rearrange("(o p) m -> p o m", p=128))
            nc.gpsimd.dma_start(xall[:, :, SC:2 * SC], xT_t[:, :, SC:2 * SC])
            nc.gpsimd.dma_start(wk_sb[:], Wk.rearrange("(o p) m -> p o m", p=128))
            nc.gpsimd.dma_start(xall[:, :, 2 * SC:3 * SC], xT_t[:, :, 2 * SC:3 * SC])
            nc.gpsimd.dma_start(wq_sb[:], Wq.rearrange("(o p) m -> p o m", p=128))
            nc.gpsimd.dma_start(xall[:, :, 3 * SC:4 * SC], xT_t[:, :, 3 * SC:4 * SC])

            ones_t = keep.tile([128, NKB, HLOC], BF16)
            nc.vector.memset(ones_t[:], 1.0)
            nc.vector.tensor_copy(vx[:, :, :, DH], ones_t[:])

            def v_group(sc, sb):
                ps = p1ps.tile([128, DLOC], F32, tag="p1")
                lo = sc * SC + sb * 128
                for dt_i in range(NDT):
                    nc.tensor.matmul(
                        ps[:], xall[:, dt_i, lo:lo + 128], wv_sb[:, dt_i, :],
                        start=(dt_i == 0), stop=(dt_i == NDT - 1))
                nc.vector.tensor_copy(
                    vx[:, sc * (SC // 128) + sb, :, 0:DH],
                    ps[:].rearrange("p (h d) -> p h d", h=HLOC))

            def kq_group(w_sb, dsts, m, sc):
                ps = p1ps.tile([128, SC], F32, tag="p1")
                ss = slice(sc * SC, (sc + 1) * SC)
                for dt_i in range(NDT):
                    nc.tensor.matmul(
                        ps[:], w_sb[:, dt_i, m * 128:(m + 1) * 128],
                        xall[:, dt_i, ss],
                        start=(dt_i == 0), stop=(dt_i == NDT - 1))
                nc.vector.tensor_copy(dsts[m][:, ss], ps[:])

            # ---- prefix: V (all s), then K/Q for head-pair 0 -------------
            with nc.named_scope("pre"):
                for sc in range(S // SC):
                    for sb in range(SC // 128):
                        v_group(sc, sb)
                for sc in range(S // SC):
                    kq_group(wk_sb, kts, 0, sc)
                for sc in range(S // SC):
                    kq_group(wq_sb, qts, 0, sc)

            # ---- attention, with next head-pair's K/Q threaded in --------
            with nc.named_scope("attn"):
                for hp in range(NM):
                    feed = []
                    if hp + 1 < NM:
                        for sc in range(S // SC):
                            feed.append(lambda m=hp + 1, s=sc:
                                        kq_group(wk_sb, kts, m, s))
                            feed.append(lambda m=hp + 1, s=sc:
                                        kq_group(wq_sb, qts, m, s))
                    for qc in range(S // QC):
                        qs = slice(qc * QC, (qc + 1) * QC)
                        pvs = [ps_pv.tile([DH + 1, QC], F32, tag=f"pv{h}",
                                          name=f"pv{h}") for h in range(2)]
                        exs = [None] * NKB

                        def emit_pv(kb):
                            for h in range(2):
                                nc.tensor.matmul(
                                    pvs[h][:], vx[:, kb, 2 * hp + h, :],
                                    exs[kb][:, h, :],
                                    start=(kb == 0), stop=(kb == NKB - 1),
                                    skip_group_check=True)

                        for kb in range(NKB):
                            ks = slice(kb * 128, (kb + 1) * 128)
                            spp = ps_s.tile([128, 2, QC], F32, tag="sc",
                                            name=f"sp{kb % 2}")
                            for h in range(2):
                                nc.tensor.matmul(
                                    spp[:, h, :],
                                    kts[hp][64 * h:64 * h + 64, ks],
                                    qts[hp][64 * h:64 * h + 64, qs],
                                    start=True, stop=True,
                                    tile_position=(64 * h, 0))
                            if kb > 0:
                                emit_pv(kb - 1)
                            ex = p2e.tile([128, 2, QC], BF16, tag="ex",
                                          name=f"ex{kb % 4}")
                            nc.scalar.activation(ex[:], spp[:], EXPF,
                                                 scale=1.0 / H)
                            exs[kb] = ex
                            if (qc * NKB + kb) % 8 == 7 and feed:
                                feed.pop(0)()
                        emit_pv(NKB - 1)

                        for h in range(2):
                            dr = p2n.tile([1, QC], F32, tag="dr", name="dr")
                            nc.vector.tensor_copy(dr[:], pvs[h][DH:DH + 1, :])
                            den = p2n.tile([1, QC], F32, tag="den", name="den")
                            nc.vector.reciprocal_approx_fast(den[:], dr[:])
                            bc = p2n.tile([DH, QC], F32, tag="bc", name="bc")
                            nc.gpsimd.partition_broadcast(bc[:], den[:])
                            nc.vector.tensor_mul(
                                ot[64 * h:64 * h + 64, hp, qs],
                                pvs[h][0:DH, :], bc[:])
                    nc.gpsimd.dma_start(out_t[:, hp, :], ot[:, hp, :])

    nc.compile()
    return nc


def run(inputs, trace=False):
    x = np.asarray(inputs["encoder_input"], dtype=np.float32)
    Wq = np.asarray(inputs["Wq"], dtype=np.float32)
    Wk = np.asarray(inputs["Wk"], dtype=np.float32)
    Wv = np.asarray(inputs["Wv"], dtype=np.float32)

    nc = _build()
    bf = ml_dtypes.bfloat16
    in_maps = []
    for c in range(NCORES):
        b, g = c // 2, c % 2
        cols = slice(g * DLOC, (g + 1) * DLOC)
        in_maps.append({
            "xT": np.ascontiguousarray(x[b].T).astype(bf),
            "Wq": np.ascontiguousarray(Wq[:, cols]).astype(bf),
            "Wk": np.ascontiguousarray(Wk[:, cols]).astype(bf),
            "Wv": np.ascontiguousarray(Wv[:, cols]).astype(bf),
        })
    res = run_bass_kernel_spmd(nc, in_maps, core_ids=list(range(NCORES)),
                               trace=trace)
    out = np.empty((B, S, D), dtype=np.float32)
    for c in range(NCORES):
        b, g = c // 2, c % 2
        out[b, :, g * DLOC:(g + 1) * DLOC] = res.results[c]["outT"].T
    return out, res


def kernel(**inputs):
    out, _ = run(inputs, trace=False)
    return out


# revision 11
# speedup vs baseline: 1.1068x; 1.0517x over previous
"""Multi-head self-attention (B=4, S=2048, D=1024, H=16) on 8 trn2 NeuronCores.

Sharding: core c -> batch b = c//2, head-group g = c%2 (8 heads, 512 of the
1024 output/QKV columns). Each core computes Q/K/V projections for its slice
and full attention for its 8 heads. Host does layout prep (x transpose + bf16
cast, W column slices) and the final gather/transpose - no collectives needed.

All matmuls in bf16 (psum accumulation f32): full PE rate, half the weight-load
time and DMA of f32r, and lower PE power draw (the f32r version tripped the HW
utilization throttle to ~54% duty).

Single fused pipeline per core (projections threaded into attention):
  prefix: V for all s (packed into Vx[128,16,8,65] bf16 with a ones column
          per head for the PV denominator), then K/Q for head-pair 0.
  per head-pair hp, per q-chunk(512): software-pipelined over 16 k-blocks:
      scoresT[k,q] psum[128,1024] <- KT-tile.T @ QT-chunk (2 heads, one bank
        each, tile_position rows 0/64 - the pair executes concurrently);
      one ACT exp over both banks -> ex bf16 [128,1024];
      pv[65,512] psum += Vx-tile.T @ ex-half (row 64 = denominator), issued
        one k-block behind the scores so PE overlaps ACT;
      every 8th iteration: one K/Q projection group for head-pair hp+1 is
        emitted (PE fills its exp-stall slack; ACT never starves).
    normalize: out = pv[0:64] * partition_broadcast(recip(pv[64])).
  output: outT[512,2048] f32 per core; host writes out[b,:,cols] = outT.T.
"""
import numpy as np
import ml_dtypes

import concourse.bacc as bacc
import concourse.mybir as mybir
import concourse.tile as tile
from concourse.bass_utils import run_bass_kernel_spmd

B, S, D, H = 4, 2048, 1024, 16
DH = D // H            # 64
NCORES = 8
HLOC = H // 2          # 8 heads per core
DLOC = HLOC * DH       # 512 output cols per core
NM = DLOC // 128       # 4 head-pair blocks
F32 = mybir.dt.float32
BF16 = mybir.dt.bfloat16
EXPF = mybir.ActivationFunctionType.Exp

SC = 512               # s-chunk for projections
QC = 512               # q-chunk in attention
NKB = S // 128         # 16 k-blocks
NDT = D // 128         # 8 contraction tiles for QKV


def _build():
    nc = bacc.Bacc("TRN2", target_bir_lowering=False, debug=False, num_devices=NCORES)
    xT = nc.dram_tensor("xT", [D, S], BF16, kind="ExternalInput").ap()
    Wq = nc.dram_tensor("Wq", [D, DLOC], BF16, kind="ExternalInput").ap()
    Wk = nc.dram_tensor("Wk", [D, DLOC], BF16, kind="ExternalInput").ap()
    Wv = nc.dram_tensor("Wv", [D, DLOC], BF16, kind="ExternalInput").ap()
    out = nc.dram_tensor("outT", [DLOC, S], F32, kind="ExternalOutput").ap()

    xT_t = xT.rearrange("(o p) s -> p o s", p=128)        # [128, 8, 2048]
    out_t = out.rearrange("(o p) s -> p o s", p=128)      # [128, 4, 2048]

    with tile.TileContext(nc) as tc:
        with tc.tile_pool(name="persist", bufs=1) as keep, \
             tc.tile_pool(name="p2e", bufs=4) as p2e, \
             tc.tile_pool(name="p2n", bufs=2) as p2n, \
             tc.tile_pool(name="p1ps", bufs=2, space="PSUM") as p1ps, \
             tc.tile_pool(name="ps_s", bufs=2, space="PSUM") as ps_s, \
             tc.tile_pool(name="ps_pv", bufs=1, space="PSUM") as ps_pv:
            qts = [keep.tile([128, S], BF16, name=f"qt{m}") for m in range(NM)]
            kts = [keep.tile([128, S], BF16, name=f"kt{m}") for m in range(NM)]
            vx = keep.tile([128, NKB, HLOC, DH + 1], BF16)
            ot = keep.tile([128, NM, S], F32)
            wq_sb = keep.tile([128, NDT, DLOC], BF16)
            wk_sb = keep.tile([128, NDT, DLOC], BF16)
            wv_sb = keep.tile([128, NDT, DLOC], BF16)
            xall = keep.tile([128, NDT, S], BF16)

            # DMAs spread across engine queues so x chunks and weights
            # transfer in parallel (first V group needs x chunk 0 + Wv only).
            nc.gpsimd.dma_start(xall[:, :, 0:SC], xT_t[:, :, 0:SC])
            nc.sync.dma_start(wv_sb[:], Wv.rearrange("(o p) m -> p o m", p=128))
            nc.scalar.dma_start(wk_sb[:], Wk.rearrange("(o p) m -> p o m", p=128))
            nc.sync.dma_start(wq_sb[:], Wq.rearrange("(o p) m -> p o m", p=128))
            nc.gpsimd.dma_start(xall[:, :, SC:2 * SC], xT_t[:, :, SC:2 * SC])
            nc.gpsimd.dma_start(xall[:, :, 2 * SC:3 * SC], xT_t[:, :, 2 * SC:3 * SC])
            nc.gpsimd.dma_start(xall[:, :, 3 * SC:4 * SC], xT_t[:, :, 3 * SC:4 * SC])

            ones_t = keep.tile([128, NKB, HLOC], BF16)
            nc.vector.memset(ones_t[:], 1.0)
            nc.vector.tensor_copy(vx[:, :, :, DH], ones_t[:])

            def v_group(sc, sb):
                ps = p1ps.tile([128, DLOC], F32, tag="p1")
                lo = sc * SC + sb * 128
                for dt_i in range(NDT):
                    nc.tensor.matmul(
                        ps[:], xall[:, dt_i, lo:lo + 128], wv_sb[:, dt_i, :],
                        start=(dt_i == 0), stop=(dt_i == NDT - 1))
                nc.vector.tensor_copy(
                    vx[:, sc * (SC // 128) + sb, :, 0:DH],
                    ps[:].rearrange("p (h d) -> p h d", h=HLOC))

            def kq_group(w_sb, dsts, m, sc):
                ps = p1ps.tile([128, SC], F32, tag="p1")
                ss = slice(sc * SC, (sc + 1) * SC)
                for dt_i in range(NDT):
                    nc.tensor.matmul(
                        ps[:], w_sb[:, dt_i, m * 128:(m + 1) * 128],
                        xall[:, dt_i, ss],
                        start=(dt_i == 0), stop=(dt_i == NDT - 1))
                nc.vector.tensor_copy(dsts[m][:, ss], ps[:])

            # ---- prefix: V (all s), K for head-pair 0, Q0 chunk 0 --------
            with nc.named_scope("pre"):
                for sc in range(S // SC):
                    for sb in range(SC // 128):
                        v_group(sc, sb)
                for sc in range(S // SC):
                    kq_group(wk_sb, kts, 0, sc)
                kq_group(wq_sb, qts, 0, 0)

            # ---- attention, with next head-pair's K/Q threaded in --------
            with nc.named_scope("attn"):
                for hp in range(NM):
                    feed = []
                    if hp == 0:
                        # rest of Q0: chunk q is consumed by q-block q, and
                        # feed slot f fires at iteration 5f+4 < 16q.
                        for sc in range(1, S // SC):
                            feed.append(lambda s=sc:
                                        kq_group(wq_sb, qts, 0, s))
                    if hp + 1 < NM:
                        for sc in range(S // SC):
                            feed.append(lambda m=hp + 1, s=sc:
                                        kq_group(wk_sb, kts, m, s))
                            feed.append(lambda m=hp + 1, s=sc:
                                        kq_group(wq_sb, qts, m, s))
                    for qc in range(S // QC):
                        qs = slice(qc * QC, (qc + 1) * QC)
                        pvs = [ps_pv.tile([DH + 1, QC], F32, tag=f"pv{h}",
                                          name=f"pv{h}") for h in range(2)]
                        exs = [None] * NKB

                        def emit_pv(kb):
                            for h in range(2):
                                nc.tensor.matmul(
                                    pvs[h][:], vx[:, kb, 2 * hp + h, :],
                                    exs[kb][:, h, :],
                                    start=(kb == 0), stop=(kb == NKB - 1),
                                    skip_group_check=True)

                        for kb in range(NKB):
                            ks = slice(kb * 128, (kb + 1) * 128)
                            spp = ps_s.tile([128, 2, QC], F32, tag="sc",
                                            name=f"sp{kb % 2}")
                            for h in range(2):
                                nc.tensor.matmul(
                                    spp[:, h, :],
                                    kts[hp][64 * h:64 * h + 64, ks],
                                    qts[hp][64 * h:64 * h + 64, qs],
                                    start=True, stop=True,
                                    tile_position=(64 * h, 0))
                            if kb > 0:
                                emit_pv(kb - 1)
                            ex = p2e.tile([128, 2, QC], BF16, tag="ex",
                                          name=f"ex{kb % 4}")
                            nc.scalar.activation(ex[:], spp[:], EXPF,
                                                 scale=1.0 / H)
                            exs[kb] = ex
                            if (qc * NKB + kb) % 5 == 4 and feed:
                                feed.pop(0)()
                        emit_pv(NKB - 1)

                        # Copy pv out of PSUM right away (frees the bank for
                        # the next q-block), normalize from the SBUF copy.
                        for h in range(2):
                            dr = p2n.tile([1, QC], F32, tag="dr", name="dr")
                            nc.vector.tensor_copy(dr[:], pvs[h][DH:DH + 1, :])
                            pvc = p2n.tile([DH, QC], F32, tag=f"pvc{h}",
                                           name=f"pvc{h}")
                            nc.vector.tensor_copy(pvc[:], pvs[h][0:DH, :])
                            den = p2n.tile([1, QC], F32, tag="den", name="den")
                            nc.vector.reciprocal_approx_fast(den[:], dr[:])
                            bc = p2n.tile([DH, QC], F32, tag="bc", name="bc")
                            nc.gpsimd.partition_broadcast(bc[:], den[:])
                            nc.vector.tensor_mul(
                                ot[64 * h:64 * h + 64, hp, qs],
                                pvc[:], bc[:])
                    nc.gpsimd.dma_start(out_t[:, hp, :], ot[:, hp, :])

    nc.compile()
    return nc


def run(inputs, trace=False):
    x = np.asarray(inputs["encoder_input"], dtype=np.float32)
    Wq = np.asarray(inputs["Wq"], dtype=np.float32)
    Wk = np.asarray(inputs["Wk"], dtype=np.float32)
    Wv = np.asarray(inputs["Wv"], dtype=np.float32)

    nc = _build()
    bf = ml_dtypes.bfloat16
    in_maps = []
    for c in range(NCORES):
        b, g = c // 2, c % 2
        cols = slice(g * DLOC, (g + 1) * DLOC)
        in_maps.append({
            "xT": np.ascontiguousarray(x[b].T).astype(bf),
            "Wq": np.ascontiguousarray(Wq[:, cols]).astype(bf),
            "Wk": np.ascontiguousarray(Wk[:, cols]).astype(bf),
            "Wv": np.ascontiguousarray(Wv[:, cols]).astype(bf),
        })
    res = run_bass_kernel_spmd(nc, in_maps, core_ids=list(range(NCORES)),
                               trace=trace)
    out = np.empty((B, S, D), dtype=np.float32)
    for c in range(NCORES):
        b, g = c // 2, c % 2
        out[b, :, g * DLOC:(g + 1) * DLOC] = res.results[c]["outT"].T
    return out, res


def kernel(**inputs):
    out, _ = run(inputs, trace=False)
    return out
